# revision 14
# baseline (speedup 1.0000x reference)
"""BinaryTreeLSTM (left-branching) Trainium2 Bass kernel.

Reference computation (per batch element):
    h0 = x[:, 0]; c0 = 0
    for t in 1..L-1:
        s = [h; x_t] @ W + b                  # W: [2D, 5D], gates i,f1,f2,o,g
        c = sig(f1)*c + sig(f2)*0 + sig(i)*tanh(g)   # f2 gate is dead (c2=0)
        h = sig(o)*tanh(c)
    out = concat([x, stack(h_1..h_{L-1})], axis=1)   # [B, 2L-1, D]

Strategy: data-parallel over batch B=64 -> 8 cores x 8 batch. Per core the
scan is sequential (1023 steps). Layout keeps the gate dimension on SBUF/PSUM
partitions so elementwise work is [128, 2, 8] tiles:

  - PSUM [128, 8banks, TG=64 steps, 8 batch]: bank m = (gate, half) m-tile,
    gate order [f1, g, i, o] (f2 dropped). x_t @ W_x + b for a 64-step group
    is precomputed INTO psum by matmuls (start=True), and the per-step
    h @ W_h matmuls accumulate on top (start=False) -- bias and leaf
    contributions cost nothing in the sequential scan.
  - Per step: 16 (ldweights+matmul) [128x128] w/ N=8, then
    ACT sigmoid/tanh reads psum bank-pairs directly, DVE does the c/h chain.
  - h is cast to bf16 for the next matmul rhs (weights bf16 -> FWL fast
    weight loads); psum accumulation and the c/h chain stay fp32.
"""

import math

import numpy as np
import ml_dtypes

import concourse.bass as bass
import concourse.mybir as mybir
from concourse.tile import TileContext

P = 128
DIM = 256
NB = 8  # batch per core
N_CORES = 8
# gate order in psum banks: [f1, g, i, o]; original W column-block indices
# (W columns are [i, f1, f2, o, g] blocks of 256)
GATE_ORIG = [1, 4, 0, 3]

F32 = mybir.dt.float32
BF16 = mybir.dt.bfloat16


def build_nc(L=1024, TG=64, dt_mm=BF16, nb=NB, with_bias=False, tg_use=60):
    """Build the Bass module for seq length L. Returns nc.

    Layout v2: psum [P, 8 banks, TG=64 slots, nb]; slots 0..tg_use-1 hold the
    per-step gate accumulators, slots 62/63 of banks 0-1 hold the fp32 c-state
    (ping-pong by global step parity) so tanh(c) gets the fast psum-src ACT
    path. h lives directly in the bf16 output buffer H_sb and feeds the next
    step's matmul rhs from there.
    """
    S = L - 1  # number of scan steps
    n_groups = math.ceil(S / tg_use)
    assert TG * nb == 512  # bank m <-> m-tile alignment requires one bank per m
    assert tg_use <= TG - 2  # slots 62/63 reserved for the c-state scratch

    nc = bass.Bass()

    # DRAM I/O (per core shapes)
    xT = nc.declare_dram_parameter("xT", [2, P, L, nb], dt_mm, isOutput=False)  # [k,d,t,b]
    wh = nc.declare_dram_parameter("wh", [2, 8, P, P], dt_mm, isOutput=False)  # [k,m,kd,md]
    wx = nc.declare_dram_parameter("wx", [2, 8, P, P], dt_mm, isOutput=False)
    bb = None
    if with_bias:
        bb = nc.declare_dram_parameter("bb", [1, 8, P], F32, isOutput=False)  # [1,m,md]
    out = nc.declare_dram_parameter("out", [P, S, 2, nb], dt_mm, isOutput=True)

    Sigmoid = mybir.ActivationFunctionType.Sigmoid
    Tanh = mybir.ActivationFunctionType.Tanh

    with TileContext(nc) as tc:
        with (
            tc.tile_pool(name="const", bufs=1) as cpool,
            tc.tile_pool(name="xin", bufs=2) as xpool,
            tc.tile_pool(name="hout", bufs=2) as hpool,
            tc.tile_pool(name="gates", bufs=2) as gpool,
            tc.tile_pool(name="psum", bufs=1, space="PSUM") as ppool,
        ):
            # --- constants ---
            wh_sb = cpool.tile([P, 2, 8, P], dt_mm, tag="wh")
            nc.sync.dma_start(wh_sb[:], wh.rearrange("k m kd md -> kd k m md"))
            wx_sb = cpool.tile([P, 2, 8, P], dt_mm, tag="wx")
            nc.sync.dma_start(wx_sb[:], wx.rearrange("k m kd md -> kd k m md"))
            if with_bias:
                # bias via a normal K=128 matmul: lhsT has b in partition 0,
                # zeros elsewhere; rhs is a ones-row (row 0) zero elsewhere.
                b_lhsT = cpool.tile([P, 8, P], F32, tag="bb")
                nc.vector.memset(b_lhsT[:], 0.0)
                nc.sync.dma_start(b_lhsT[0:1, :, :], bb[:])
                ones_row = cpool.tile([P, TG, nb], F32, tag="ones")
                nc.vector.memset(ones_row[:], 0.0)
                nc.vector.memset(ones_row[0:1, :, :], 1.0)

            # --- psum: 8 banks exactly; bank m <-> m-tile (gate, half) ---
            psum_t = ppool.tile([P, 8, TG, nb], F32, tag="ps")

            # --- initial state ---
            x0 = cpool.tile([P, 2, 1, nb], dt_mm, tag="x0")
            nc.sync.dma_start(x0[:], xT[:, :, 0:1, :].rearrange("k d t b -> d k t b"))
            h0_sb = cpool.tile([P, 2, nb], dt_mm, tag="h0")
            nc.vector.tensor_copy(h0_sb[:], x0[:, :, 0, :])
            # c for "step -1" lives in scratch slot parity 1 (step 0 reads it)
            nc.vector.memset(psum_t[:, 0:2, TG - 2 + 1, :], 0.0)

            rhs_prev = (h0_sb[:, 0, :], h0_sb[:, 1, :])

            for g in range(n_groups):
                s0 = g * tg_use
                tg = min(tg_use, S - s0)
                # leaves consumed by steps s0..s0+tg-1 are x[:, s0+1 .. s0+tg]
                x_sb = xpool.tile([P, 2, tg_use, nb], dt_mm, tag="x")
                nc.sync.dma_start(
                    x_sb[:, :, :tg, :],
                    xT[:, :, s0 + 1 : s0 + 1 + tg, :].rearrange("k d t b -> d k t b"),
                )
                H_sb = hpool.tile([P, tg_use, 2, nb], dt_mm, tag="H")

                # --- precompute x_t @ W_x (+ b) into psum for the group ---
                for m in range(8):
                    dst = psum_t[:, m, :tg, :]
                    for k in range(2):
                        nc.tensor.matmul(
                            dst,
                            wx_sb[:, k, m, :],
                            x_sb[:, k, :tg, :],
                            start=(k == 0),
                            stop=False,
                            skip_group_check=True,
                        )
                    if with_bias:
                        nc.tensor.matmul(
                            dst,
                            b_lhsT[:, m, :],
                            ones_row[:, :tg, :],
                            start=False,
                            stop=False,
                            skip_group_check=True,
                        )

                # --- sequential scan ---
                for tau in range(tg):
                    sg = s0 + tau  # global step index
                    c_new = psum_t[:, 0:2, TG - 2 + (sg % 2), :]
                    c_old = psum_t[:, 0:2, TG - 2 + ((sg + 1) % 2), :]
                    for m in range(8):
                        for k in range(2):
                            nc.tensor.matmul(
                                psum_t[:, m, tau, :],
                                wh_sb[:, k, m, :],
                                rhs_prev[k],
                                start=False,
                                stop=(k == 1),
                                skip_group_check=True,
                            )
                    # banks: f1=0:2, g=2:4, i=4:6, o=6:8
                    sig_f1 = gpool.tile([P, 2, nb], F32, tag="sf1")
                    nc.scalar.activation(sig_f1[:], psum_t[:, 0:2, tau, :], Sigmoid)
                    cf = gpool.tile([P, 2, nb], F32, tag="cf")
                    nc.vector.tensor_mul(cf[:], sig_f1[:], c_old)
                    tanh_g = gpool.tile([P, 2, nb], F32, tag="tg")
                    nc.scalar.activation(tanh_g[:], psum_t[:, 2:4, tau, :], Tanh)
                    sig_io = gpool.tile([P, 4, nb], F32, tag="sio")
                    nc.scalar.activation(sig_io[:], psum_t[:, 4:8, tau, :], Sigmoid)
                    tmp = gpool.tile([P, 2, nb], F32, tag="tmp")
                    nc.vector.tensor_mul(tmp[:], sig_io[:, 0:2, :], tanh_g[:])
                    nc.vector.tensor_add(c_new, cf[:], tmp[:])
                    tanh_c = gpool.tile([P, 2, nb], F32, tag="tc")
                    nc.scalar.activation(tanh_c[:], c_new, Tanh)
                    # h halves straight into the bf16 output buffer; half 0
                    # first so the next step's k0 matmuls can start early
                    nc.vector.tensor_mul(
                        H_sb[:, tau, 0, :], sig_io[:, 2, :], tanh_c[:, 0, :]
                    )
                    nc.vector.tensor_mul(
                        H_sb[:, tau, 1, :], sig_io[:, 3, :], tanh_c[:, 1, :]
                    )
                    rhs_prev = (H_sb[:, tau, 0, :], H_sb[:, tau, 1, :])

                nc.sync.dma_start(out[:, s0 : s0 + tg, :, :], H_sb[:, :tg, :, :])

    _legalize_matmul_waits(nc)
    return nc


def _legalize_matmul_waits(nc):
    """Walrus codegen on trn2 accepts only ONE sync wait on compute/DMA
    instruction structs (S3_LW, S3S3D3_TT, PSEUDO_DMA_DIRECT2D, ...) and TWO
    on CTRL_NO ones (NoOp, Drain). Spill extra waits onto preceding NoOps."""
    exempt = (
        mybir.InstUnconditionalBranch,
        mybir.InstCall,
        mybir.InstEventSemaphore,
        mybir.InstHalt,
    )
    fn = nc.m.functions[0]
    for blk in fn.blocks:
        out = []
        for inst in blk.instructions:
            si = inst.sync_info
            cap = 1
            if (
                not isinstance(inst, exempt)
                and si is not None
                and si.on_wait
                and len(si.on_wait) > cap
            ):
                extra = list(si.on_wait[:-cap])
                si.on_wait = list(si.on_wait[-cap:])
                for w in extra:
                    nop = mybir.InstNoOp(
                        name=nc.get_next_instruction_name(), ins=[], outs=[]
                    )
                    nop.engine = inst.engine
                    nop.sync_info = mybir.SyncInfo(on_wait=[w], on_update=[])
                    nc.register_instruction(nop)
                    out.append(nop)
            out.append(inst)
        blk.instructions[:] = out


def prep_weights(W, b, dt_np=ml_dtypes.bfloat16):
    """W [2D, 5D] f32, b [5D] f32 -> (wh [2,8,P,P], wx [2,8,P,P], bb [1,8,P])."""
    D = DIM
    Wre = np.asarray(W).reshape(2 * D, 5, D)
    cols = np.concatenate([Wre[:, o, :] for o in GATE_ORIG], axis=1)  # [512, 1024]
    wh_full, wx_full = cols[:D], cols[D:]

    def tile4(w):  # [256, 1024] -> [k, m, kd, md]
        return np.ascontiguousarray(
            w.reshape(2, P, 8, P).transpose(0, 2, 1, 3)
        ).astype(dt_np)

    bre = np.asarray(b).reshape(5, D)[GATE_ORIG].reshape(8, P)  # [m, md]
    bb = np.ascontiguousarray(bre[None]).astype(np.float32)  # [1, 8, P]
    return tile4(wh_full), tile4(wx_full), bb


def prep_x_shard(x_shard, dt_np=ml_dtypes.bfloat16):
    """x_shard [nb, L, D] f32 -> xT [2, P, L, nb]."""
    nb, L, D = x_shard.shape
    return np.ascontiguousarray(
        np.asarray(x_shard).transpose(2, 1, 0).reshape(2, P, L, nb)
    ).astype(dt_np)


def unpack_out(out_core):
    """out [P, S, 2, nb] (any float dtype) -> internal [nb, S, D] fp32."""
    Pp, S, two, nb = out_core.shape
    return (
        np.ascontiguousarray(out_core.transpose(3, 1, 2, 0))
        .reshape(nb, S, DIM)
        .astype(np.float32)
    )


_NC_CACHE = {}

# test hooks: set _TRACE=True before calling kernel() to capture a profile;
# the BassKernelResults lands in LAST_RESULTS.
_TRACE = False
LAST_RESULTS = None


def _get_nc(L, TG=64, dt_mm=BF16, with_bias=False):
    key = (L, TG, str(dt_mm), with_bias)
    if key not in _NC_CACHE:
        _NC_CACHE[key] = build_nc(L=L, TG=TG, dt_mm=dt_mm, with_bias=with_bias)
    return _NC_CACHE[key]


def kernel(x, W, b, lengths=None, **_ignored):
    """Full inputs -> full output [B, 2L-1, D]. Distributes over 8 cores."""
    from concourse.bass_utils import run_bass_kernel_spmd

    x = np.asarray(x, dtype=np.float32)
    B, L, D = x.shape
    assert D == DIM and B % N_CORES == 0
    nb = B // N_CORES
    S = L - 1

    with_bias = bool(np.any(np.asarray(b)))
    nc = _get_nc(L, with_bias=with_bias)
    wh, wx, bb = prep_weights(W, b)
    in_maps = []
    for j in range(N_CORES):
        xTj = prep_x_shard(x[j * nb : (j + 1) * nb])
        m = {"xT": xTj, "wh": wh, "wx": wx}
        if with_bias:
            m["bb"] = bb
        in_maps.append(m)

    global LAST_RESULTS
    kr = run_bass_kernel_spmd(nc, in_maps, list(range(N_CORES)), trace=_TRACE)
    LAST_RESULTS = kr
    res = kr.results

    internal = np.empty((B, S, D), dtype=np.float32)
    for j in range(N_CORES):
        internal[j * nb : (j + 1) * nb] = unpack_out(res[j]["out"])
    return np.concatenate([x, internal], axis=1)
